# revision 53
# baseline (speedup 1.0000x reference)
"""Trainium2 Bass kernel for nn_Attention_82815559401482 (sparse_attention).

Full-input contract: kernel(**inputs) takes the complete (unsharded) inputs
and returns the full [16, 784, 512] output. Internally shards data-parallel
over the batch dim across 8 NeuronCores (2 batches per core), builds one SPMD
Bass/Tile program, and runs it via run_bass_kernel_spmd.

Math (per core, b in {0,1} local batches):
  qkv = BN(x @ w_qkv^T)           -> folded into w/b on host, q pre-scaled
  S^T[key,q] = k·q + bias         -> bias applied multiplicatively post-exp:
  E = exp(S^T_raw) * exp(bias)    (exp(bias) precomputed on host, fp16)
  U = V^T-weighted sums: U[d,q] = sum_k v[k,d] E[k,q]   (fp16 matmul)
  Z[q] = sum_k E[k,q]             (ones-matmul, output replicated over 128 p)
  O^T = U/Z + bv ; hardswish ; proj with folded BN (+ /6 folded into w_proj)

Perf structure (v2):
  - stage 2 processes HEAD PAIRS (2g, 2g+1): the two K=32 score matmuls are
    issued back-to-back at tile_position (0/64,0) and (32/96,0) so they run
    concurrently in distinct 32-row PE bands (row tiling).
  - exp covers both heads' score banks in one ACT instruction (pair tiles),
    halving the ~300-cycle per-instruction ACT overhead. Same for the DVE
    bias-multiply and the normalize chain.
  - exp(bias) table is stored partition-contiguous on host and kept fully
    SBUF-resident (all 8 heads), DMA'd once on the gpsimd queue.
  - U/Z matmuls are software-pipelined one chunk behind S so the PE never
    sits behind the ACT/DVE chain.
"""

import os
import sys

import numpy as np


def _ensure_deps():
    try:
        import concourse.bass  # noqa: F401
        return
    except ImportError:
        pass
    for p in ("/opt/trn_rl_repo", "/root/.axon_site/_ro/trn_rl_repo"):
        if os.path.isdir(p) and p not in sys.path:
            sys.path.insert(0, p)
    import concourse.bass  # noqa: F401


_ensure_deps()

import ml_dtypes  # noqa: E402,F401
import concourse.bass as bass  # noqa: E402
import concourse.mybir as mybir  # noqa: E402
import concourse.tile as tile  # noqa: E402
from concourse.alu_op_type import AluOpType  # noqa: E402
from concourse.vector_clock import ScopedClock  # noqa: E402
from concourse.bass_utils import run_bass_kernel_spmd  # noqa: E402
from contextlib import ExitStack  # noqa: E402


def _patch_tile_drain():
    """The installed walrus rejects >1 semaphore wait on one SP CTRL
    instruction ("Too many sync wait commands"); TileContext's tail drain
    puts one wait per live semaphore on a single Drain. Split the waits
    across dedicated nop instructions instead."""
    if getattr(tile.TileContext, "_drain_patched", False):
        return

    def _drain_and_barrier(self, tick_clock, wait_clock):
        nc = self.nc
        drain_inst = nc.sync.drain()
        wait_clock.add_sem_waits(
            drain_inst.ins, ScopedClock({None: tick_clock.global_clock})
        )
        si = drain_inst.ins.sync_info
        waits = list(si.on_wait or [])
        if len(waits) > 1:
            si.on_wait.clear()
            for w in waits:
                w_inst = nc.sync.nop(nofuse=True, hint="drain_wait")
                w_inst.ins.sync_info = mybir.SyncInfo(on_wait=[w], on_update=[])
        nc.all_engine_barrier()
        assert self.sems is not None
        popped = nc._tile_sem_poison_stack.pop()
        assert popped is self._sem_poison
        nc.clear_and_free_semaphores(list(self.sems.allocated().values()))
        nc.all_engine_barrier()

    tile.TileContext._drain_and_barrier = _drain_and_barrier
    tile.TileContext._drain_patched = True


_patch_tile_drain()


def _split_multi_waits(nc):
    """This walrus build rejects instructions carrying more than one
    semaphore wait ("Too many sync wait commands"). Hoist extra waits onto
    same-engine nop instructions inserted just before the instruction."""
    n = 0
    for fn in nc.m.functions:
        for blk in fn.blocks:
            new_insts = []
            for inst in blk.instructions:
                si = inst.sync_info
                if si is not None and si.on_wait and len(si.on_wait) > 1:
                    waits = list(si.on_wait)
                    for i, w in enumerate(waits[1:]):
                        nop = mybir.InstNoOp(
                            name=f"{inst.name}_xw{i}",
                            engine=inst.engine,
                            bass_nofuse=True,
                            sync_info=mybir.SyncInfo(on_wait=[w], on_update=[]),
                        )
                        new_insts.append(nop)
                        n += 1
                    si.on_wait.clear()
                    si.on_wait.append(waits[0])
                new_insts.append(inst)
            blk.instructions.clear()
            blk.instructions.extend(new_insts)
    return n


# Problem dims (hardcoded per contract)
B, RES, DIM = 16, 28, 512
N = RES * RES  # 784
H, KD = 8, 32
D = 128  # v head dim
DH = D * H  # 1024
EPS = 1e-5
SCALE = KD ** -0.5

NCORES = 8
BPC = B // NCORES  # 2 batches per core
T = BPC * N  # 1568 tokens per core

FP = mybir.dt.float32
BF = mybir.dt.bfloat16
FH = mybir.dt.float16

KCH = [(i * 128, min(128, N - i * 128)) for i in range((N + 127) // 128)]  # 7
NKC = len(KCH)
QBL = [(0, 392), (392, 392)]  # query free-dim blocks within 784
TB4 = [(i * 392, 392) for i in range(4)]  # token blocks of 1568 (392 each)
DIMC = DIM // 128  # 4
DHC = DH // 128  # 8

AFT = mybir.ActivationFunctionType

_PROGRAM_CACHE = {}


def build_program():
    nc = bass.Bass("TRN2", target_bir_lowering=False, debug=False,
                   num_devices=NCORES)

    xT = nc.dram_tensor("xT", [DIM, T], FH, kind="ExternalInput").ap()
    wqkT = nc.dram_tensor("wqkT", [DIM, 512], FH, kind="ExternalInput").ap()
    wvT = nc.dram_tensor("wvT", [DIM, DH], FH, kind="ExternalInput").ap()
    wpT = nc.dram_tensor("wpT", [DH, DIM], FH, kind="ExternalInput").ap()
    bqk = nc.dram_tensor("bqk", [512], FP, kind="ExternalInput").ap()
    bvrow = nc.dram_tensor("bvrow", [128, DH], FH, kind="ExternalInput").ap()
    bp = nc.dram_tensor("bp", [DIM], FP, kind="ExternalInput").ap()
    # exp(bias), partition-contiguous, [p, qb, head-pair g, chunk, j*392+q']
    ebp = nc.dram_tensor("ebp", [128, 8 * NKC * N], FH,
                         kind="ExternalInput").ap()
    out = nc.dram_tensor("out", [DIM, T], FH, kind="ExternalOutput").ap()

    with tile.TileContext(nc) as tc, ExitStack() as ctx:
        # ---------- persistent pools ----------
        wpool = ctx.enter_context(tc.tile_pool(name="w", bufs=1))
        cpool = ctx.enter_context(tc.tile_pool(name="consts", bufs=1))

        wqk_sb = wpool.tile([128, DIMC, 512], FH, tag="wqk")
        bqk_sb = cpool.tile([128, DIMC], FP, tag="bqk")
        wv_sb = wpool.tile([128, DIMC, DH], FH, tag="wv")
        bvrow_sb = cpool.tile([128, DHC, 128], FH, tag="bvrow")
        eb_sb = wpool.tile([128, 8, NKC, N], FH, tag="eb")
        wp_sb = wpool.tile([128, DHC, 512], FH, tag="wp")
        bp_sb = cpool.tile([128, DIMC], FP, tag="bp")
        ones_sb = cpool.tile([128, 128], FH, tag="ones")

        # qk^T activations: [32-row band, m-chunk, token]; m-chunk 0: q heads
        # 0-3 (head h%4 at partitions 32*(h%4)), 1: q heads 4-7, 2/3: same for k
        qkT_sb = wpool.tile([128, 4, T], FH, tag="qkT")
        # v [tokens, (b,kc), head, dim]
        v_sb = wpool.tile([128, BPC * NKC, H, 128], FH, tag="vsb")
        # O^T [vdim, head, token]
        o_sb = wpool.tile([128, DHC, T], FH, tag="osb")

        # ---------- stage 1: qkv projection ----------
        with tc.tile_pool(name="s1", bufs=2) as s1pool, \
             tc.tile_pool(name="ps1", bufs=1, space="PSUM") as ps1:
            # x first (critical path for stage 1), on the sync queue; split
            # by token block so the first qk matmuls start ~4x earlier
            xT_sb = s1pool.tile([128, DIMC, T], FH, tag="xT", bufs=1)
            for c in range(DIMC):
                nc.sync.dma_start(wqk_sb[:, c, :],
                                  wqkT[c * 128:(c + 1) * 128, :])
            nc.sync.dma_start(bqk_sb[:, :],
                              bqk.rearrange("(c p) -> p c", p=128))
            for (no, nn) in TB4:
                for c in range(DIMC):
                    nc.sync.dma_start(xT_sb[:, c, no:no + nn],
                                      xT[c * 128:(c + 1) * 128, no:no + nn])
            for c in range(DIMC):
                nc.sync.dma_start(wv_sb[:, c, :],
                                  wvT[c * 128:(c + 1) * 128, :])
            nc.sync.dma_start(bvrow_sb[:, :, :],
                              bvrow.rearrange("p (c d) -> p c d", c=DHC))
            # big exp-bias table: all heads resident; sync queue, after the
            # stage-1-critical loads so it doesn't delay them
            nc.sync.dma_start(
                eb_sb[:, :, :, :].rearrange("p c h q -> p (c h q)"), ebp)
            for c in range(DHC):
                nc.scalar.dma_start(wp_sb[:, c, :],
                                    wpT[c * 128:(c + 1) * 128, :])
            nc.sync.dma_start(bp_sb[:, :], bp.rearrange("(c p) -> p c", p=128))
            nc.vector.memset(ones_sb[:, :], 1.0)

            # warm up the GpSimd ucode (first call pays ~6us table load)
            gwarm = cpool.tile([128, 8], FH, tag="gwarm")
            nc.vector.memset(gwarm[:, :], 0.0)
            nc.gpsimd.tensor_tensor(gwarm[:, :], gwarm[:, :], gwarm[:, :],
                                    op=AluOpType.mult)

            # q/k: out [128 ch, token-block]; token-block outer so compute
            # starts after the first quarter of x lands
            for (no, nn) in TB4:
                for mc in range(4):
                    qk_ps = ps1.tile([128, 392], FP, tag="qkps", bufs=2)
                    for c in range(DIMC):
                        nc.tensor.matmul(
                            qk_ps[:, :nn],
                            lhsT=wqk_sb[:, c, mc * 128:(mc + 1) * 128],
                            rhs=xT_sb[:, c, no:no + nn],
                            start=(c == 0), stop=(c == DIMC - 1))
                    nc.scalar.activation(qkT_sb[:, mc, no:no + nn],
                                         qk_ps[:, :nn], AFT.Identity,
                                         bias=bqk_sb[:, mc:mc + 1])

            # v: out [token-chunk, v-channel-block]
            for b in range(BPC):
                for kc, (ko, kn) in enumerate(KCH):
                    to = b * N + ko
                    for nb in range(2):
                        v_ps = ps1.tile([128, 512], FP, tag="vps", bufs=2)
                        for c in range(DIMC):
                            nc.tensor.matmul(
                                v_ps[:kn, :],
                                lhsT=xT_sb[:, c, to:to + kn],
                                rhs=wv_sb[:, c, nb * 512:(nb + 1) * 512],
                                start=(c == 0), stop=(c == DIMC - 1))
                        nc.vector.tensor_tensor(
                            v_sb[:kn, b * NKC + kc, nb * 4:(nb + 1) * 4, :],
                            v_ps[:kn, :],
                            bvrow_sb[:kn, nb * 4:(nb + 1) * 4, :],
                            op=AluOpType.add)

        # ---------- stage 2: attention, head pairs, global pipeline ----------
        # Per chunk-step: two row-tiled concurrent S matmuls (K=32, PE bands
        # 0/32 or 64/96) into a double-buffered 2-bank pair tile; one pair
        # exp (ACT); one in-place pair bias-mult (DVE, flat views for 2x
        # mode). Z matmuls trail by ZLAG chunk-steps, U by ULAG, so the PE
        # never waits on the ACT->DVE chain; recip/umult are emitted as soon
        # as their block's last Z/U is, releasing psum banks early.
        with tc.tile_pool(name="s2", bufs=2) as s2pool, \
             tc.tile_pool(name="ps2", bufs=1, space="PSUM") as ps2:
            blocks = [(b, qo, qn, g)
                      for b in range(BPC) for (qo, qn) in QBL
                      for g in range(4)]
            uz_state = {}

            def s_mms(bi, i):
                b, qo, qn, g = blocks[bi]
                to = b * N
                ko, kn = KCH[i]
                h0 = 2 * g
                s_t = ps2.tile([128, 2, 512], FP, tag="s", bufs=2,
                               name=f"s_{bi}_{i}")
                for j in range(2):
                    h = h0 + j
                    hp = 32 * (h % 4)
                    hq = h // 4
                    hk = 2 + h // 4
                    nc.tensor.matmul(
                        s_t[:kn, j, :qn],
                        lhsT=qkT_sb[hp:hp + 32, hk, to + ko:to + ko + kn],
                        rhs=qkT_sb[hp:hp + 32, hq, to + qo:to + qo + qn],
                        start=True, stop=True, tile_position=(hp, 0))
                e_t = s2pool.tile([128, 2, 392], FH, tag="e",
                                  bufs=3, name=f"e_{bi}_{i}")
                nc.scalar.activation(e_t[:kn, :, :qn],
                                     s_t[:kn, :, :qn], AFT.Exp)
                return e_t

            def e2_mult(bi, i, e_t):
                b, qo, qn, g = blocks[bi]
                kn = KCH[i][1]
                # multiply by exp(bias); flat contiguous views keep the DVE
                # in 2x packed mode. High priority so the scheduler never
                # starves U/Z of e2 behind per-block finish work.
                qb4g = (qo // 392) * 4 + g
                e2_t = s2pool.tile([128, 2, 392], FH, tag="e2",
                                   bufs=ULAG + 2, name=f"e2_{bi}_{i}")
                # chunk 3 multiplies on the otherwise-idle GpSimd
                eng = nc.gpsimd if i == 3 else nc.vector
                with tc.high_priority(offset=60):
                    eng.tensor_tensor(
                        e2_t[:kn].rearrange("p b q -> p (b q)"),
                        e_t[:kn].rearrange("p b q -> p (b q)"),
                        eb_sb[:kn, qb4g, i, :],
                        op=AluOpType.mult)
                return e2_t

            def z_sstep(bi, i, e2_t):
                b, qo, qn, g = blocks[bi]
                kn = KCH[i][1]
                if i == 0:
                    uz_state[bi] = {
                        "z": ps2.tile([128, 2, 512], FP, tag="z", bufs=1,
                                      name=f"z_{bi}")}
                z_ps = uz_state[bi]["z"]
                for j in range(2):
                    nc.tensor.matmul(
                        z_ps[:, j, :qn],
                        lhsT=ones_sb[:kn, :],
                        rhs=e2_t[:kn, j, :qn],
                        start=(i == 0), stop=(i == NKC - 1))
                if i == NKC - 1:
                    # reciprocal right away: releases the z banks fast
                    r_t = s2pool.tile([128, 2, 392], FP, tag="r", bufs=2,
                                      name=f"r_{bi}")
                    nc.vector.reciprocal_approx_fast(r_t[:, :, :qn],
                                                     z_ps[:, :, :qn])
                    uz_state[bi]["r"] = r_t

            def u_sstep(bi, i, e2_t):
                b, qo, qn, g = blocks[bi]
                kn = KCH[i][1]
                if i == 0:
                    uz_state[bi]["u"] = ps2.tile([128, 2, 512], FP, tag="u",
                                                 bufs=1, name=f"u_{bi}")
                u_ps = uz_state[bi]["u"]
                for j in range(2):
                    h = 2 * g + j
                    nc.tensor.matmul(
                        u_ps[:, j, :qn],
                        lhsT=v_sb[:kn, b * NKC + i, h, :],
                        rhs=e2_t[:kn, j, :qn],
                        start=(i == 0), stop=(i == NKC - 1))
                if i == NKC - 1:
                    finish_block(bi)

            def finish_block(bi):
                b, qo, qn, g = blocks[bi]
                to = b * N
                h0 = 2 * g
                st = uz_state.pop(bi)
                u_ps, r_t = st["u"], st["r"]
                d_t = s2pool.tile([128, 2, 392], FH, tag="d", bufs=1,
                                  name=f"d_{bi}")
                nc.vector.tensor_tensor(d_t[:, :, :qn], u_ps[:, :, :qn],
                                        r_t[:, :, :qn], op=AluOpType.mult)
                # hardswish: clip on DVE (4x dual-scalar), final mul on GpSimd
                t_t = s2pool.tile([128, 2, 392], FH, tag="t", bufs=1,
                                  name=f"t_{bi}")
                nc.vector.tensor_scalar(t_t[:, :, :qn], d_t[:, :, :qn],
                                        3.0, 0.0,
                                        op0=AluOpType.add, op1=AluOpType.max)
                a_t = s2pool.tile([128, 2, 392], FH, tag="a", bufs=1,
                                  name=f"a_{bi}")
                nc.vector.tensor_scalar(a_t[:, :, :qn], t_t[:, :, :qn],
                                        6.0, None, op0=AluOpType.min)
                nc.gpsimd.tensor_tensor(
                    o_sb[:, h0:h0 + 2, to + qo:to + qo + qn],
                    a_t[:, :, :qn], d_t[:, :, :qn], op=AluOpType.mult)

            # Global pipeline over per-chunk steps.
            ZLAG, ULAG = 4, 6
            seq = [(bi, i) for bi in range(len(blocks))
                   for i in range(NKC)]
            e2s = {}
            for t in range(len(seq) + ULAG):
                if t < len(seq):
                    bi, i = seq[t]
                    e_t = s_mms(bi, i)
                if t - ZLAG >= 0 and t - ZLAG < len(seq):
                    z_sstep(*seq[t - ZLAG], e2s[t - ZLAG])
                if t < len(seq):
                    e2s[t] = e2_mult(bi, i, e_t)
                if t - ULAG >= 0:
                    u_sstep(*seq[t - ULAG], e2s.pop(t - ULAG))

        # ---------- stage 3: output projection ----------
        with tc.tile_pool(name="s3", bufs=2) as s3pool, \
             tc.tile_pool(name="ps3", bufs=1, space="PSUM") as ps3:
            for (no, nn) in TB4:
                pj = [ps3.tile([128, 392], FP, tag=f"pj{c4}", bufs=1,
                               name=f"pj{c4}_{no}")
                      for c4 in range(DIMC)]
                for dhc in range(DHC):
                    for c4 in range(DIMC):
                        nc.tensor.matmul(
                            pj[c4][:, :nn],
                            lhsT=wp_sb[:, dhc, c4 * 128:(c4 + 1) * 128],
                            rhs=o_sb[:, dhc, no:no + nn],
                            start=(dhc == 0), stop=(dhc == DHC - 1))
                for c4 in range(DIMC):
                    o_st = s3pool.tile([128, 392], FH, tag="outst", bufs=4)
                    nc.scalar.activation(o_st[:, :nn], pj[c4][:, :nn],
                                         AFT.Identity,
                                         bias=bp_sb[:, c4:c4 + 1])
                    nc.sync.dma_start(out[c4 * 128:(c4 + 1) * 128, no:no + nn],
                                      o_st[:, :nn])

    # populate .instr bytes for InstISA (custom-DVE ops) — raw Bass skips this
    mybir.codegen_inst_isa_subclasses(nc)
    nsplit = _split_multi_waits(nc)
    if os.environ.get("KERNEL_DEBUG"):
        print(f"[kernel] split {nsplit} multi-wait instructions")
    return nc


def _prepare_host_inputs(x, w_qkv, qkv_g, qkv_b, qkv_m, qkv_v, ab, w_proj,
                         proj_g, proj_b, proj_m, proj_v, bias_idx):
    f32 = np.float32
    x = np.asarray(x, f32)
    w_qkv = np.asarray(w_qkv, f32)
    qkv_g = np.asarray(qkv_g, f32)
    qkv_b = np.asarray(qkv_b, f32)
    qkv_m = np.asarray(qkv_m, f32)
    qkv_v = np.asarray(qkv_v, f32)
    ab = np.asarray(ab, f32)
    w_proj = np.asarray(w_proj, f32)
    proj_g = np.asarray(proj_g, f32)
    proj_b = np.asarray(proj_b, f32)
    proj_m = np.asarray(proj_m, f32)
    proj_v = np.asarray(proj_v, f32)
    bias_idx = np.asarray(bias_idx)

    # fold qkv BN: y = (x@W^T)*s + (b - m*s)
    s = qkv_g / np.sqrt(qkv_v + EPS)
    w_f = w_qkv * s[:, None]
    b_f = qkv_b - qkv_m * s

    # channel c = h*192 + i; i<32 q (pre-scale by SCALE), <64 k, else v
    q_rows = [w_f[h * 192:h * 192 + 32] * SCALE for h in range(H)]
    k_rows = [w_f[h * 192 + 32:h * 192 + 64] for h in range(H)]
    v_rows = [w_f[h * 192 + 64:h * 192 + 192] for h in range(H)]
    q_b = [b_f[h * 192:h * 192 + 32] * SCALE for h in range(H)]
    k_b = [b_f[h * 192 + 32:h * 192 + 64] for h in range(H)]
    v_b = [b_f[h * 192 + 64:h * 192 + 192] for h in range(H)]

    w_qk = np.concatenate(q_rows + k_rows, axis=0)      # [512, 512]
    bqk = np.concatenate(q_b + k_b, axis=0)             # [512]
    w_v = np.concatenate(v_rows, axis=0)                # [1024, 512]
    bv = np.concatenate(v_b, axis=0)                    # [1024]

    wqkT = np.ascontiguousarray(w_qk.T)                 # [512 dim, 512 ch]
    wvT = np.ascontiguousarray(w_v.T)                   # [512, 1024]

    # fold proj BN + hardswish /6: P = hs6(o) @ (W*s/6)^T + (b - m*s)
    sp = proj_g / np.sqrt(proj_v + EPS)
    w_p = w_proj * sp[:, None] / 6.0
    bpv = proj_b - proj_m * sp
    wpT = np.ascontiguousarray(w_p.T)                   # [1024, 512]

    # multiplicative positional bias exp(ab[h, bias_idx]), fp16,
    # partition-contiguous: ebp[p, (qb, g, kc, j, q')]
    ebias = np.zeros((H, NKC * 128, N), np.float16)  # pad to 896 key rows
    ebias[:, :N, :] = np.exp(ab[:, bias_idx]).astype(np.float16)
    # [g, j, kc, p, qb, q'] -> [p, qb, g, kc, j, q']
    ebp = np.ascontiguousarray(
        ebias.reshape(4, 2, NKC, 128, 2, 392).transpose(3, 4, 0, 2, 1, 5)
        .reshape(128, 8 * NKC * N))

    wqkT = wqkT.astype(np.float16)
    wvT = wvT.astype(np.float16)
    wpT = wpT.astype(np.float16)

    # x transposed: [B, DIM, N] then per-core concat of its 2 batches
    xT_all = np.ascontiguousarray(x.transpose(0, 2, 1).astype(np.float16))

    in_maps = []
    for c in range(NCORES):
        xt_core = np.ascontiguousarray(
            np.concatenate([xT_all[BPC * c + b] for b in range(BPC)], axis=1))
        in_maps.append(dict(
            xT=xt_core, wqkT=wqkT, wvT=wvT, wpT=wpT,
            bqk=np.ascontiguousarray(bqk),
            bvrow=np.ascontiguousarray(
                np.broadcast_to(bv.astype(np.float16), (128, DH))),
            bp=np.ascontiguousarray(bpv), ebp=ebp,
        ))
    return in_maps


def _get_program():
    if "nc" not in _PROGRAM_CACHE:
        _PROGRAM_CACHE["nc"] = build_program()
    return _PROGRAM_CACHE["nc"]


def run(inputs: dict, trace: bool = False, trace_kwargs: dict | None = None):
    """Build+run; returns (full_output [16,784,512], BassKernelResults)."""
    nc = _get_program()
    in_maps = _prepare_host_inputs(**inputs)
    kw = {}
    if trace:
        kw = dict(trace=True, trace_cores=[0], **(trace_kwargs or {}))
    res = run_bass_kernel_spmd(nc, in_maps, core_ids=list(range(NCORES)), **kw)
    outs = []
    for c in range(NCORES):
        o = res.results[c]["out"]  # [512, 1568] fp16
        o = o.reshape(DIM, BPC, N).transpose(1, 2, 0)  # [2, 784, 512]
        outs.append(o)
    full = np.concatenate(outs, axis=0).astype(np.float32)
    return full, res


def kernel(**inputs) -> np.ndarray:
    out, _ = run(inputs, trace=False)
    return out


# revision 54
# speedup vs baseline: 1.0045x; 1.0045x over previous
"""Trainium2 Bass kernel for nn_Attention_82815559401482 (sparse_attention).

Full-input contract: kernel(**inputs) takes the complete (unsharded) inputs
and returns the full [16, 784, 512] output. Internally shards data-parallel
over the batch dim across 8 NeuronCores (2 batches per core), builds one SPMD
Bass/Tile program, and runs it via run_bass_kernel_spmd.

Math (per core, b in {0,1} local batches):
  qkv = BN(x @ w_qkv^T)           -> folded into w/b on host, q pre-scaled
  S^T[key,q] = k·q + bias         -> bias applied multiplicatively post-exp:
  E = exp(S^T_raw) * exp(bias)    (exp(bias) precomputed on host, fp16)
  U = V^T-weighted sums: U[d,q] = sum_k v[k,d] E[k,q]   (fp16 matmul)
  Z[q] = sum_k E[k,q]             (ones-matmul, output replicated over 128 p)
  O^T = U/Z + bv ; hardswish ; proj with folded BN (+ /6 folded into w_proj)

Perf structure (v2):
  - stage 2 processes HEAD PAIRS (2g, 2g+1): the two K=32 score matmuls are
    issued back-to-back at tile_position (0/64,0) and (32/96,0) so they run
    concurrently in distinct 32-row PE bands (row tiling).
  - exp covers both heads' score banks in one ACT instruction (pair tiles),
    halving the ~300-cycle per-instruction ACT overhead. Same for the DVE
    bias-multiply and the normalize chain.
  - exp(bias) table is stored partition-contiguous on host and kept fully
    SBUF-resident (all 8 heads), DMA'd once on the gpsimd queue.
  - U/Z matmuls are software-pipelined one chunk behind S so the PE never
    sits behind the ACT/DVE chain.
"""

import os
import sys

import numpy as np


def _ensure_deps():
    try:
        import concourse.bass  # noqa: F401
        return
    except ImportError:
        pass
    for p in ("/opt/trn_rl_repo", "/root/.axon_site/_ro/trn_rl_repo"):
        if os.path.isdir(p) and p not in sys.path:
            sys.path.insert(0, p)
    import concourse.bass  # noqa: F401


_ensure_deps()

import ml_dtypes  # noqa: E402,F401
import concourse.bass as bass  # noqa: E402
import concourse.mybir as mybir  # noqa: E402
import concourse.tile as tile  # noqa: E402
from concourse.alu_op_type import AluOpType  # noqa: E402
from concourse.vector_clock import ScopedClock  # noqa: E402
from concourse.bass_utils import run_bass_kernel_spmd  # noqa: E402
from contextlib import ExitStack  # noqa: E402


def _patch_tile_drain():
    """The installed walrus rejects >1 semaphore wait on one SP CTRL
    instruction ("Too many sync wait commands"); TileContext's tail drain
    puts one wait per live semaphore on a single Drain. Split the waits
    across dedicated nop instructions instead."""
    if getattr(tile.TileContext, "_drain_patched", False):
        return

    def _drain_and_barrier(self, tick_clock, wait_clock):
        nc = self.nc
        drain_inst = nc.sync.drain()
        wait_clock.add_sem_waits(
            drain_inst.ins, ScopedClock({None: tick_clock.global_clock})
        )
        si = drain_inst.ins.sync_info
        waits = list(si.on_wait or [])
        if len(waits) > 1:
            si.on_wait.clear()
            for w in waits:
                w_inst = nc.sync.nop(nofuse=True, hint="drain_wait")
                w_inst.ins.sync_info = mybir.SyncInfo(on_wait=[w], on_update=[])
        nc.all_engine_barrier()
        assert self.sems is not None
        popped = nc._tile_sem_poison_stack.pop()
        assert popped is self._sem_poison
        nc.clear_and_free_semaphores(list(self.sems.allocated().values()))
        nc.all_engine_barrier()

    tile.TileContext._drain_and_barrier = _drain_and_barrier
    tile.TileContext._drain_patched = True


_patch_tile_drain()


def _split_multi_waits(nc):
    """This walrus build rejects instructions carrying more than one
    semaphore wait ("Too many sync wait commands"). Hoist extra waits onto
    same-engine nop instructions inserted just before the instruction."""
    n = 0
    for fn in nc.m.functions:
        for blk in fn.blocks:
            new_insts = []
            for inst in blk.instructions:
                si = inst.sync_info
                if si is not None and si.on_wait and len(si.on_wait) > 1:
                    waits = list(si.on_wait)
                    for i, w in enumerate(waits[1:]):
                        nop = mybir.InstNoOp(
                            name=f"{inst.name}_xw{i}",
                            engine=inst.engine,
                            bass_nofuse=True,
                            sync_info=mybir.SyncInfo(on_wait=[w], on_update=[]),
                        )
                        new_insts.append(nop)
                        n += 1
                    si.on_wait.clear()
                    si.on_wait.append(waits[0])
                new_insts.append(inst)
            blk.instructions.clear()
            blk.instructions.extend(new_insts)
    return n


# Problem dims (hardcoded per contract)
B, RES, DIM = 16, 28, 512
N = RES * RES  # 784
H, KD = 8, 32
D = 128  # v head dim
DH = D * H  # 1024
EPS = 1e-5
SCALE = KD ** -0.5

NCORES = 8
BPC = B // NCORES  # 2 batches per core
T = BPC * N  # 1568 tokens per core

FP = mybir.dt.float32
BF = mybir.dt.bfloat16
FH = mybir.dt.float16

KCH = [(i * 128, min(128, N - i * 128)) for i in range((N + 127) // 128)]  # 7
NKC = len(KCH)
QBL = [(0, 392), (392, 392)]  # query free-dim blocks within 784
TB4 = [(i * 392, 392) for i in range(4)]  # token blocks of 1568 (392 each)
DIMC = DIM // 128  # 4
DHC = DH // 128  # 8

AFT = mybir.ActivationFunctionType

_PROGRAM_CACHE = {}


def build_program():
    nc = bass.Bass("TRN2", target_bir_lowering=False, debug=False,
                   num_devices=NCORES)

    xT = nc.dram_tensor("xT", [DIM, T], FH, kind="ExternalInput").ap()
    wqkT = nc.dram_tensor("wqkT", [DIM, 512], FH, kind="ExternalInput").ap()
    wvT = nc.dram_tensor("wvT", [DIM, DH], FH, kind="ExternalInput").ap()
    wpT = nc.dram_tensor("wpT", [DH, DIM], FH, kind="ExternalInput").ap()
    bqk = nc.dram_tensor("bqk", [512], FP, kind="ExternalInput").ap()
    bvrow = nc.dram_tensor("bvrow", [128, DH], FH, kind="ExternalInput").ap()
    bp = nc.dram_tensor("bp", [DIM], FP, kind="ExternalInput").ap()
    # exp(bias), partition-contiguous, [p, qb, head-pair g, chunk, j*392+q']
    ebp = nc.dram_tensor("ebp", [128, 8 * NKC * N], FH,
                         kind="ExternalInput").ap()
    out = nc.dram_tensor("out", [DIM, T], FH, kind="ExternalOutput").ap()

    with tile.TileContext(nc) as tc, ExitStack() as ctx:
        # ---------- persistent pools ----------
        wpool = ctx.enter_context(tc.tile_pool(name="w", bufs=1))
        cpool = ctx.enter_context(tc.tile_pool(name="consts", bufs=1))

        wqk_sb = wpool.tile([128, DIMC, 512], FH, tag="wqk")
        bqk_sb = cpool.tile([128, DIMC], FP, tag="bqk")
        wv_sb = wpool.tile([128, DIMC, DH], FH, tag="wv")
        bvrow_sb = cpool.tile([128, DHC, 128], FH, tag="bvrow")
        eb_sb = wpool.tile([128, 8, NKC, N], FH, tag="eb")
        wp_sb = wpool.tile([128, DHC, 512], FH, tag="wp")
        bp_sb = cpool.tile([128, DIMC], FP, tag="bp")
        ones_sb = cpool.tile([128, 128], FH, tag="ones")

        # qk^T activations: [32-row band, m-chunk, token]; m-chunk 0: q heads
        # 0-3 (head h%4 at partitions 32*(h%4)), 1: q heads 4-7, 2/3: same for k
        qkT_sb = wpool.tile([128, 4, T], FH, tag="qkT")
        # v [tokens, (b,kc), head, dim]
        v_sb = wpool.tile([128, BPC * NKC, H, 128], FH, tag="vsb")
        # O^T [vdim, head, token]
        o_sb = wpool.tile([128, DHC, T], FH, tag="osb")

        # ---------- stage 1: qkv projection ----------
        with tc.tile_pool(name="s1", bufs=2) as s1pool, \
             tc.tile_pool(name="ps1", bufs=1, space="PSUM") as ps1:
            # x first (critical path for stage 1), on the sync queue; split
            # by token block so the first qk matmuls start ~4x earlier
            xT_sb = s1pool.tile([128, DIMC, T], FH, tag="xT", bufs=1)
            for c in range(DIMC):
                nc.sync.dma_start(wqk_sb[:, c, :],
                                  wqkT[c * 128:(c + 1) * 128, :])
            nc.sync.dma_start(bqk_sb[:, :],
                              bqk.rearrange("(c p) -> p c", p=128))
            for (no, nn) in TB4:
                for c in range(DIMC):
                    nc.sync.dma_start(xT_sb[:, c, no:no + nn],
                                      xT[c * 128:(c + 1) * 128, no:no + nn])
            for c in range(DIMC):
                nc.sync.dma_start(wv_sb[:, c, :],
                                  wvT[c * 128:(c + 1) * 128, :])
            nc.sync.dma_start(bvrow_sb[:, :, :],
                              bvrow.rearrange("p (c d) -> p c d", c=DHC))
            # big exp-bias table: all heads resident; sync queue, after the
            # stage-1-critical loads so it doesn't delay them
            nc.sync.dma_start(
                eb_sb[:, :, :, :].rearrange("p c h q -> p (c h q)"), ebp)
            for c in range(DHC):
                nc.scalar.dma_start(wp_sb[:, c, :],
                                    wpT[c * 128:(c + 1) * 128, :])
            nc.sync.dma_start(bp_sb[:, :], bp.rearrange("(c p) -> p c", p=128))
            nc.vector.memset(ones_sb[:, :], 1.0)

            # warm up the GpSimd ucode (first call pays ~6us table load)
            gwarm = cpool.tile([128, 8], FH, tag="gwarm")
            nc.vector.memset(gwarm[:, :], 0.0)
            nc.gpsimd.tensor_tensor(gwarm[:, :], gwarm[:, :], gwarm[:, :],
                                    op=AluOpType.mult)

            # q/k: out [128 ch, token-block]; token-block outer so compute
            # starts after the first quarter of x lands
            for (no, nn) in TB4:
                for mc in range(4):
                    qk_ps = ps1.tile([128, 392], FP, tag="qkps", bufs=2)
                    for c in range(DIMC):
                        nc.tensor.matmul(
                            qk_ps[:, :nn],
                            lhsT=wqk_sb[:, c, mc * 128:(mc + 1) * 128],
                            rhs=xT_sb[:, c, no:no + nn],
                            start=(c == 0), stop=(c == DIMC - 1))
                    nc.scalar.activation(qkT_sb[:, mc, no:no + nn],
                                         qk_ps[:, :nn], AFT.Identity,
                                         bias=bqk_sb[:, mc:mc + 1])

            # v: out [token-chunk, v-channel-block]
            for b in range(BPC):
                for kc, (ko, kn) in enumerate(KCH):
                    to = b * N + ko
                    for nb in range(2):
                        v_ps = ps1.tile([128, 512], FP, tag="vps", bufs=2)
                        for c in range(DIMC):
                            nc.tensor.matmul(
                                v_ps[:kn, :],
                                lhsT=xT_sb[:, c, to:to + kn],
                                rhs=wv_sb[:, c, nb * 512:(nb + 1) * 512],
                                start=(c == 0), stop=(c == DIMC - 1))
                        nc.vector.tensor_tensor(
                            v_sb[:kn, b * NKC + kc, nb * 4:(nb + 1) * 4, :],
                            v_ps[:kn, :],
                            bvrow_sb[:kn, nb * 4:(nb + 1) * 4, :],
                            op=AluOpType.add)

        # ---------- stage 2: attention, head pairs, global pipeline ----------
        # Per chunk-step: two row-tiled concurrent S matmuls (K=32, PE bands
        # 0/32 or 64/96) into a double-buffered 2-bank pair tile; one pair
        # exp (ACT); one in-place pair bias-mult (DVE, flat views for 2x
        # mode). Z matmuls trail by ZLAG chunk-steps, U by ULAG, so the PE
        # never waits on the ACT->DVE chain; recip/umult are emitted as soon
        # as their block's last Z/U is, releasing psum banks early.
        with tc.tile_pool(name="s2", bufs=2) as s2pool, \
             tc.tile_pool(name="ps2", bufs=1, space="PSUM") as ps2:
            blocks = [(b, qo, qn, g)
                      for b in range(BPC) for (qo, qn) in QBL
                      for g in range(4)]
            uz_state = {}

            def s_mms(bi, i):
                b, qo, qn, g = blocks[bi]
                to = b * N
                ko, kn = KCH[i]
                h0 = 2 * g
                s_t = ps2.tile([128, 2, 512], FP, tag="s", bufs=2,
                               name=f"s_{bi}_{i}")
                for j in range(2):
                    h = h0 + j
                    hp = 32 * (h % 4)
                    hq = h // 4
                    hk = 2 + h // 4
                    nc.tensor.matmul(
                        s_t[:kn, j, :qn],
                        lhsT=qkT_sb[hp:hp + 32, hk, to + ko:to + ko + kn],
                        rhs=qkT_sb[hp:hp + 32, hq, to + qo:to + qo + qn],
                        start=True, stop=True, tile_position=(hp, 0))
                e_t = s2pool.tile([128, 2, 392], FH, tag="e",
                                  bufs=3, name=f"e_{bi}_{i}")
                nc.scalar.activation(e_t[:kn, :, :qn],
                                     s_t[:kn, :, :qn], AFT.Exp)
                return e_t

            def e2_mult(bi, i, e_t):
                b, qo, qn, g = blocks[bi]
                kn = KCH[i][1]
                # multiply by exp(bias); flat contiguous views keep the DVE
                # in 2x packed mode. High priority so the scheduler never
                # starves U/Z of e2 behind per-block finish work.
                qb4g = (qo // 392) * 4 + g
                e2_t = s2pool.tile([128, 2, 392], FH, tag="e2",
                                   bufs=ULAG + 2, name=f"e2_{bi}_{i}")
                with tc.high_priority(offset=60):
                    nc.vector.tensor_tensor(
                        e2_t[:kn].rearrange("p b q -> p (b q)"),
                        e_t[:kn].rearrange("p b q -> p (b q)"),
                        eb_sb[:kn, qb4g, i, :],
                        op=AluOpType.mult)
                return e2_t

            def z_sstep(bi, i, e2_t):
                b, qo, qn, g = blocks[bi]
                kn = KCH[i][1]
                if i == 0:
                    uz_state[bi] = {
                        "z": ps2.tile([128, 2, 512], FP, tag="z", bufs=1,
                                      name=f"z_{bi}")}
                z_ps = uz_state[bi]["z"]
                for j in range(2):
                    nc.tensor.matmul(
                        z_ps[:, j, :qn],
                        lhsT=ones_sb[:kn, :],
                        rhs=e2_t[:kn, j, :qn],
                        start=(i == 0), stop=(i == NKC - 1))
                if i == NKC - 1:
                    # reciprocal right away: releases the z banks fast
                    r_t = s2pool.tile([128, 2, 392], FP, tag="r", bufs=2,
                                      name=f"r_{bi}")
                    nc.vector.reciprocal_approx_fast(r_t[:, :, :qn],
                                                     z_ps[:, :, :qn])
                    uz_state[bi]["r"] = r_t

            def u_sstep(bi, i, e2_t):
                b, qo, qn, g = blocks[bi]
                kn = KCH[i][1]
                if i == 0:
                    uz_state[bi]["u"] = ps2.tile([128, 2, 512], FP, tag="u",
                                                 bufs=1, name=f"u_{bi}")
                u_ps = uz_state[bi]["u"]
                for j in range(2):
                    h = 2 * g + j
                    nc.tensor.matmul(
                        u_ps[:, j, :qn],
                        lhsT=v_sb[:kn, b * NKC + i, h, :],
                        rhs=e2_t[:kn, j, :qn],
                        start=(i == 0), stop=(i == NKC - 1))
                if i == NKC - 1:
                    finish_block(bi)

            def finish_block(bi):
                b, qo, qn, g = blocks[bi]
                to = b * N
                h0 = 2 * g
                st = uz_state.pop(bi)
                u_ps, r_t = st["u"], st["r"]
                d_t = s2pool.tile([128, 2, 392], FH, tag="d", bufs=1,
                                  name=f"d_{bi}")
                nc.vector.tensor_tensor(d_t[:, :, :qn], u_ps[:, :, :qn],
                                        r_t[:, :, :qn], op=AluOpType.mult)
                # hardswish: clip on DVE (4x dual-scalar), final mul on GpSimd
                t_t = s2pool.tile([128, 2, 392], FH, tag="t", bufs=1,
                                  name=f"t_{bi}")
                nc.vector.tensor_scalar(t_t[:, :, :qn], d_t[:, :, :qn],
                                        3.0, 0.0,
                                        op0=AluOpType.add, op1=AluOpType.max)
                a_t = s2pool.tile([128, 2, 392], FH, tag="a", bufs=1,
                                  name=f"a_{bi}")
                nc.vector.tensor_scalar(a_t[:, :, :qn], t_t[:, :, :qn],
                                        6.0, None, op0=AluOpType.min)
                nc.gpsimd.tensor_tensor(
                    o_sb[:, h0:h0 + 2, to + qo:to + qo + qn],
                    a_t[:, :, :qn], d_t[:, :, :qn], op=AluOpType.mult)

            # Global pipeline over per-chunk steps.
            ZLAG, ULAG = 4, 6
            seq = [(bi, i) for bi in range(len(blocks))
                   for i in range(NKC)]
            e2s = {}
            for t in range(len(seq) + ULAG):
                if t < len(seq):
                    bi, i = seq[t]
                    e_t = s_mms(bi, i)
                if t - ZLAG >= 0 and t - ZLAG < len(seq):
                    z_sstep(*seq[t - ZLAG], e2s[t - ZLAG])
                if t < len(seq):
                    e2s[t] = e2_mult(bi, i, e_t)
                if t - ULAG >= 0:
                    u_sstep(*seq[t - ULAG], e2s.pop(t - ULAG))

        # ---------- stage 3: output projection ----------
        with tc.tile_pool(name="s3", bufs=2) as s3pool, \
             tc.tile_pool(name="ps3", bufs=1, space="PSUM") as ps3:
            for (no, nn) in TB4:
                pj = [ps3.tile([128, 392], FP, tag=f"pj{c4}", bufs=1,
                               name=f"pj{c4}_{no}")
                      for c4 in range(DIMC)]
                for dhc in range(DHC):
                    for c4 in range(DIMC):
                        nc.tensor.matmul(
                            pj[c4][:, :nn],
                            lhsT=wp_sb[:, dhc, c4 * 128:(c4 + 1) * 128],
                            rhs=o_sb[:, dhc, no:no + nn],
                            start=(dhc == 0), stop=(dhc == DHC - 1))
                for c4 in range(DIMC):
                    o_st = s3pool.tile([128, 392], FH, tag="outst", bufs=4)
                    nc.scalar.activation(o_st[:, :nn], pj[c4][:, :nn],
                                         AFT.Identity,
                                         bias=bp_sb[:, c4:c4 + 1])
                    nc.sync.dma_start(out[c4 * 128:(c4 + 1) * 128, no:no + nn],
                                      o_st[:, :nn])

    # populate .instr bytes for InstISA (custom-DVE ops) — raw Bass skips this
    mybir.codegen_inst_isa_subclasses(nc)
    nsplit = _split_multi_waits(nc)
    if os.environ.get("KERNEL_DEBUG"):
        print(f"[kernel] split {nsplit} multi-wait instructions")
    return nc


def _prepare_host_inputs(x, w_qkv, qkv_g, qkv_b, qkv_m, qkv_v, ab, w_proj,
                         proj_g, proj_b, proj_m, proj_v, bias_idx):
    f32 = np.float32
    x = np.asarray(x, f32)
    w_qkv = np.asarray(w_qkv, f32)
    qkv_g = np.asarray(qkv_g, f32)
    qkv_b = np.asarray(qkv_b, f32)
    qkv_m = np.asarray(qkv_m, f32)
    qkv_v = np.asarray(qkv_v, f32)
    ab = np.asarray(ab, f32)
    w_proj = np.asarray(w_proj, f32)
    proj_g = np.asarray(proj_g, f32)
    proj_b = np.asarray(proj_b, f32)
    proj_m = np.asarray(proj_m, f32)
    proj_v = np.asarray(proj_v, f32)
    bias_idx = np.asarray(bias_idx)

    # fold qkv BN: y = (x@W^T)*s + (b - m*s)
    s = qkv_g / np.sqrt(qkv_v + EPS)
    w_f = w_qkv * s[:, None]
    b_f = qkv_b - qkv_m * s

    # channel c = h*192 + i; i<32 q (pre-scale by SCALE), <64 k, else v
    q_rows = [w_f[h * 192:h * 192 + 32] * SCALE for h in range(H)]
    k_rows = [w_f[h * 192 + 32:h * 192 + 64] for h in range(H)]
    v_rows = [w_f[h * 192 + 64:h * 192 + 192] for h in range(H)]
    q_b = [b_f[h * 192:h * 192 + 32] * SCALE for h in range(H)]
    k_b = [b_f[h * 192 + 32:h * 192 + 64] for h in range(H)]
    v_b = [b_f[h * 192 + 64:h * 192 + 192] for h in range(H)]

    w_qk = np.concatenate(q_rows + k_rows, axis=0)      # [512, 512]
    bqk = np.concatenate(q_b + k_b, axis=0)             # [512]
    w_v = np.concatenate(v_rows, axis=0)                # [1024, 512]
    bv = np.concatenate(v_b, axis=0)                    # [1024]

    wqkT = np.ascontiguousarray(w_qk.T)                 # [512 dim, 512 ch]
    wvT = np.ascontiguousarray(w_v.T)                   # [512, 1024]

    # fold proj BN + hardswish /6: P = hs6(o) @ (W*s/6)^T + (b - m*s)
    sp = proj_g / np.sqrt(proj_v + EPS)
    w_p = w_proj * sp[:, None] / 6.0
    bpv = proj_b - proj_m * sp
    wpT = np.ascontiguousarray(w_p.T)                   # [1024, 512]

    # multiplicative positional bias exp(ab[h, bias_idx]), fp16,
    # partition-contiguous: ebp[p, (qb, g, kc, j, q')]
    ebias = np.zeros((H, NKC * 128, N), np.float16)  # pad to 896 key rows
    ebias[:, :N, :] = np.exp(ab[:, bias_idx]).astype(np.float16)
    # [g, j, kc, p, qb, q'] -> [p, qb, g, kc, j, q']
    ebp = np.ascontiguousarray(
        ebias.reshape(4, 2, NKC, 128, 2, 392).transpose(3, 4, 0, 2, 1, 5)
        .reshape(128, 8 * NKC * N))

    wqkT = wqkT.astype(np.float16)
    wvT = wvT.astype(np.float16)
    wpT = wpT.astype(np.float16)

    # x transposed: [B, DIM, N] then per-core concat of its 2 batches
    xT_all = np.ascontiguousarray(x.transpose(0, 2, 1).astype(np.float16))

    in_maps = []
    for c in range(NCORES):
        xt_core = np.ascontiguousarray(
            np.concatenate([xT_all[BPC * c + b] for b in range(BPC)], axis=1))
        in_maps.append(dict(
            xT=xt_core, wqkT=wqkT, wvT=wvT, wpT=wpT,
            bqk=np.ascontiguousarray(bqk),
            bvrow=np.ascontiguousarray(
                np.broadcast_to(bv.astype(np.float16), (128, DH))),
            bp=np.ascontiguousarray(bpv), ebp=ebp,
        ))
    return in_maps


def _get_program():
    if "nc" not in _PROGRAM_CACHE:
        _PROGRAM_CACHE["nc"] = build_program()
    return _PROGRAM_CACHE["nc"]


def run(inputs: dict, trace: bool = False, trace_kwargs: dict | None = None):
    """Build+run; returns (full_output [16,784,512], BassKernelResults)."""
    nc = _get_program()
    in_maps = _prepare_host_inputs(**inputs)
    kw = {}
    if trace:
        kw = dict(trace=True, trace_cores=[0], **(trace_kwargs or {}))
    res = run_bass_kernel_spmd(nc, in_maps, core_ids=list(range(NCORES)), **kw)
    outs = []
    for c in range(NCORES):
        o = res.results[c]["out"]  # [512, 1568] fp16
        o = o.reshape(DIM, BPC, N).transpose(1, 2, 0)  # [2, 784, 512]
        outs.append(o)
    full = np.concatenate(outs, axis=0).astype(np.float32)
    return full, res


def kernel(**inputs) -> np.ndarray:
    out, _ = run(inputs, trace=False)
    return out


# revision 57
# speedup vs baseline: 1.0178x; 1.0132x over previous
"""Trainium2 Bass kernel for nn_Attention_82815559401482 (sparse_attention).

Full-input contract: kernel(**inputs) takes the complete (unsharded) inputs
and returns the full [16, 784, 512] output. Internally shards data-parallel
over the batch dim across 8 NeuronCores (2 batches per core), builds one SPMD
Bass/Tile program, and runs it via run_bass_kernel_spmd.

Math (per core, b in {0,1} local batches):
  qkv = BN(x @ w_qkv^T)           -> folded into w/b on host, q pre-scaled
  S^T[key,q] = k·q + bias         -> bias applied multiplicatively post-exp:
  E = exp(S^T_raw) * exp(bias)    (exp(bias) precomputed on host, fp16)
  U = V^T-weighted sums: U[d,q] = sum_k v[k,d] E[k,q]   (fp16 matmul)
  Z[q] = sum_k E[k,q]             (ones-matmul, output replicated over 128 p)
  O^T = U/Z + bv ; hardswish ; proj with folded BN (+ /6 folded into w_proj)

Perf structure:
  - stage 2 processes HEAD PAIRS (2g, 2g+1): the two K=32 score matmuls are
    issued back-to-back at tile_position (0/64,0) and (32/96,0) so they run
    CONCURRENTLY in distinct 32-row PE bands (row tiling, ~2x on S).
  - one exp (ACT) and one bias-multiply (DVE, flat contiguous views for the
    2x packed mode) cover both heads of a chunk-step, halving the ~300-cycle
    per-instruction overheads. S psum pair tiles are double-buffered so the
    S front never waits on exp completion (lockstep kills the pipeline).
  - global software pipeline over all 112 chunk-steps: Z matmuls trail the
    S/exp/mult front by ZLAG=4 steps, U by ULAG=6, giving the ACT->DVE chain
    (~3.5us latency) slack so the PE never idles and the HAM clock gate
    stays at 2.4 GHz. recip (z release) is emitted with the block's last Z,
    u*r with the last U; e2-mult runs at elevated scheduler priority so
    per-block finish work never starves U/Z of e2.
  - hardswish: clip on DVE dual-scalar ops, final multiply on the otherwise
    idle GpSimd (its TT supports mult; tensor_scalar/min are broken there).
  - exp(bias) table is precomputed on host, fp16, partition-contiguous in
    (qb, head-pair, chunk)-major order so every DVE operand is flat, and
    kept fully SBUF-resident (all 8 heads, 86KB/partition), one DMA.
  - stage 1/3 use 392-wide token blocks (no LDW-bound 32-col runt matmuls);
    output is stored fp16 (host converts to fp32).
"""

import os
import sys

import numpy as np


def _ensure_deps():
    try:
        import concourse.bass  # noqa: F401
        return
    except ImportError:
        pass
    for p in ("/opt/trn_rl_repo", "/root/.axon_site/_ro/trn_rl_repo"):
        if os.path.isdir(p) and p not in sys.path:
            sys.path.insert(0, p)
    import concourse.bass  # noqa: F401


_ensure_deps()

import ml_dtypes  # noqa: E402,F401
import concourse.bass as bass  # noqa: E402
import concourse.mybir as mybir  # noqa: E402
import concourse.tile as tile  # noqa: E402
from concourse.alu_op_type import AluOpType  # noqa: E402
from concourse.vector_clock import ScopedClock  # noqa: E402
from concourse.bass_utils import run_bass_kernel_spmd  # noqa: E402
from contextlib import ExitStack  # noqa: E402


def _patch_tile_drain():
    """The installed walrus rejects >1 semaphore wait on one SP CTRL
    instruction ("Too many sync wait commands"); TileContext's tail drain
    puts one wait per live semaphore on a single Drain. Split the waits
    across dedicated nop instructions instead."""
    if getattr(tile.TileContext, "_drain_patched", False):
        return

    def _drain_and_barrier(self, tick_clock, wait_clock):
        nc = self.nc
        drain_inst = nc.sync.drain()
        wait_clock.add_sem_waits(
            drain_inst.ins, ScopedClock({None: tick_clock.global_clock})
        )
        si = drain_inst.ins.sync_info
        waits = list(si.on_wait or [])
        if len(waits) > 1:
            si.on_wait.clear()
            for w in waits:
                w_inst = nc.sync.nop(nofuse=True, hint="drain_wait")
                w_inst.ins.sync_info = mybir.SyncInfo(on_wait=[w], on_update=[])
        nc.all_engine_barrier()
        assert self.sems is not None
        popped = nc._tile_sem_poison_stack.pop()
        assert popped is self._sem_poison
        nc.clear_and_free_semaphores(list(self.sems.allocated().values()))
        nc.all_engine_barrier()

    tile.TileContext._drain_and_barrier = _drain_and_barrier
    tile.TileContext._drain_patched = True


_patch_tile_drain()


def _split_multi_waits(nc):
    """This walrus build rejects instructions carrying more than one
    semaphore wait ("Too many sync wait commands"). Hoist extra waits onto
    same-engine nop instructions inserted just before the instruction."""
    n = 0
    for fn in nc.m.functions:
        for blk in fn.blocks:
            new_insts = []
            for inst in blk.instructions:
                si = inst.sync_info
                if si is not None and si.on_wait and len(si.on_wait) > 1:
                    waits = list(si.on_wait)
                    for i, w in enumerate(waits[1:]):
                        nop = mybir.InstNoOp(
                            name=f"{inst.name}_xw{i}",
                            engine=inst.engine,
                            bass_nofuse=True,
                            sync_info=mybir.SyncInfo(on_wait=[w], on_update=[]),
                        )
                        new_insts.append(nop)
                        n += 1
                    si.on_wait.clear()
                    si.on_wait.append(waits[0])
                new_insts.append(inst)
            blk.instructions.clear()
            blk.instructions.extend(new_insts)
    return n


# Problem dims (hardcoded per contract)
B, RES, DIM = 16, 28, 512
N = RES * RES  # 784
H, KD = 8, 32
D = 128  # v head dim
DH = D * H  # 1024
EPS = 1e-5
SCALE = KD ** -0.5

NCORES = 8
BPC = B // NCORES  # 2 batches per core
T = BPC * N  # 1568 tokens per core

FP = mybir.dt.float32
BF = mybir.dt.bfloat16
FH = mybir.dt.float16

KCH = [(i * 128, min(128, N - i * 128)) for i in range((N + 127) // 128)]  # 7
NKC = len(KCH)
QBL = [(0, 392), (392, 392)]  # query free-dim blocks within 784
TB4 = [(i * 392, 392) for i in range(4)]  # token blocks of 1568 (392 each)
DIMC = DIM // 128  # 4
DHC = DH // 128  # 8

AFT = mybir.ActivationFunctionType

_PROGRAM_CACHE = {}


def build_program():
    nc = bass.Bass("TRN2", target_bir_lowering=False, debug=False,
                   num_devices=NCORES)

    xT = nc.dram_tensor("xT", [DIM, T], FH, kind="ExternalInput").ap()
    wqkT = nc.dram_tensor("wqkT", [DIM, 512], FH, kind="ExternalInput").ap()
    wvT = nc.dram_tensor("wvT", [DIM, DH], FH, kind="ExternalInput").ap()
    wpT = nc.dram_tensor("wpT", [DH, DIM], FH, kind="ExternalInput").ap()
    bqk = nc.dram_tensor("bqk", [512], FP, kind="ExternalInput").ap()
    bvrow = nc.dram_tensor("bvrow", [128, DH], FH, kind="ExternalInput").ap()
    bp = nc.dram_tensor("bp", [DIM], FP, kind="ExternalInput").ap()
    # exp(bias), partition-contiguous, [p, qb, head-pair g, chunk, j*392+q']
    ebp = nc.dram_tensor("ebp", [128, 8 * NKC * N], FH,
                         kind="ExternalInput").ap()
    out = nc.dram_tensor("out", [DIM, T], FH, kind="ExternalOutput").ap()

    with tile.TileContext(nc) as tc, ExitStack() as ctx:
        # ---------- persistent pools ----------
        wpool = ctx.enter_context(tc.tile_pool(name="w", bufs=1))
        cpool = ctx.enter_context(tc.tile_pool(name="consts", bufs=1))

        wqk_sb = wpool.tile([128, DIMC, 512], FH, tag="wqk")
        bqk_sb = cpool.tile([128, DIMC], FP, tag="bqk")
        wv_sb = wpool.tile([128, DIMC, DH], FH, tag="wv")
        bvrow_sb = cpool.tile([128, DHC, 128], FH, tag="bvrow")
        eb_sb = wpool.tile([128, 8, NKC, N], FH, tag="eb")
        wp_sb = wpool.tile([128, DHC, 512], FH, tag="wp")
        bp_sb = cpool.tile([128, DIMC], FP, tag="bp")
        ones_sb = cpool.tile([128, 128], FH, tag="ones")

        # qk^T activations: [32-row band, m-chunk, token]; m-chunk 0: q heads
        # 0-3 (head h%4 at partitions 32*(h%4)), 1: q heads 4-7, 2/3: same for k
        qkT_sb = wpool.tile([128, 4, T], FH, tag="qkT")
        # v [tokens, (b,kc), head, dim]
        v_sb = wpool.tile([128, BPC * NKC, H, 128], FH, tag="vsb")
        # O^T [vdim, head, token]
        o_sb = wpool.tile([128, DHC, T], FH, tag="osb")

        # ---------- stage 1: qkv projection ----------
        with tc.tile_pool(name="s1", bufs=2) as s1pool, \
             tc.tile_pool(name="ps1", bufs=1, space="PSUM") as ps1:
            # x first (critical path for stage 1), on the sync queue; split
            # by token block so the first qk matmuls start ~4x earlier
            xT_sb = s1pool.tile([128, DIMC, T], FH, tag="xT", bufs=1)
            for c in range(DIMC):
                nc.sync.dma_start(wqk_sb[:, c, :],
                                  wqkT[c * 128:(c + 1) * 128, :])
            nc.sync.dma_start(bqk_sb[:, :],
                              bqk.rearrange("(c p) -> p c", p=128))
            for (no, nn) in TB4:
                for c in range(DIMC):
                    nc.sync.dma_start(xT_sb[:, c, no:no + nn],
                                      xT[c * 128:(c + 1) * 128, no:no + nn])
            for c in range(DIMC):
                nc.sync.dma_start(wv_sb[:, c, :],
                                  wvT[c * 128:(c + 1) * 128, :])
            nc.sync.dma_start(bvrow_sb[:, :, :],
                              bvrow.rearrange("p (c d) -> p c d", c=DHC))
            # big exp-bias table: all heads resident; sync queue, after the
            # stage-1-critical loads so it doesn't delay them
            nc.sync.dma_start(
                eb_sb[:, :, :, :].rearrange("p c h q -> p (c h q)"), ebp)
            for c in range(DHC):
                nc.scalar.dma_start(wp_sb[:, c, :],
                                    wpT[c * 128:(c + 1) * 128, :])
            nc.sync.dma_start(bp_sb[:, :], bp.rearrange("(c p) -> p c", p=128))
            nc.vector.memset(ones_sb[:, :], 1.0)

            # warm up the GpSimd ucode (first call pays ~6us table load)
            gwarm = cpool.tile([128, 8], FH, tag="gwarm")
            nc.vector.memset(gwarm[:, :], 0.0)
            nc.gpsimd.tensor_tensor(gwarm[:, :], gwarm[:, :], gwarm[:, :],
                                    op=AluOpType.mult)

            # q/k: out [128 ch, token-block]; token-block outer so compute
            # starts after the first quarter of x lands
            for (no, nn) in TB4:
                for mc in range(4):
                    qk_ps = ps1.tile([128, 392], FP, tag="qkps", bufs=2)
                    for c in range(DIMC):
                        nc.tensor.matmul(
                            qk_ps[:, :nn],
                            lhsT=wqk_sb[:, c, mc * 128:(mc + 1) * 128],
                            rhs=xT_sb[:, c, no:no + nn],
                            start=(c == 0), stop=(c == DIMC - 1))
                    nc.scalar.activation(qkT_sb[:, mc, no:no + nn],
                                         qk_ps[:, :nn], AFT.Identity,
                                         bias=bqk_sb[:, mc:mc + 1])

            # v: out [token-chunk, v-channel-block]
            for b in range(BPC):
                for kc, (ko, kn) in enumerate(KCH):
                    to = b * N + ko
                    for nb in range(2):
                        v_ps = ps1.tile([128, 512], FP, tag="vps", bufs=2)
                        for c in range(DIMC):
                            nc.tensor.matmul(
                                v_ps[:kn, :],
                                lhsT=xT_sb[:, c, to:to + kn],
                                rhs=wv_sb[:, c, nb * 512:(nb + 1) * 512],
                                start=(c == 0), stop=(c == DIMC - 1))
                        nc.vector.tensor_tensor(
                            v_sb[:kn, b * NKC + kc, nb * 4:(nb + 1) * 4, :],
                            v_ps[:kn, :],
                            bvrow_sb[:kn, nb * 4:(nb + 1) * 4, :],
                            op=AluOpType.add)

        # ---------- stage 2: attention, head pairs, global pipeline ----------
        # Per chunk-step: two row-tiled concurrent S matmuls (K=32, PE bands
        # 0/32 or 64/96) into a double-buffered 2-bank pair tile; one pair
        # exp (ACT); one in-place pair bias-mult (DVE, flat views for 2x
        # mode). Z matmuls trail by ZLAG chunk-steps, U by ULAG, so the PE
        # never waits on the ACT->DVE chain; recip/umult are emitted as soon
        # as their block's last Z/U is, releasing psum banks early.
        with tc.tile_pool(name="s2", bufs=2) as s2pool, \
             tc.tile_pool(name="ps2", bufs=1, space="PSUM") as ps2:
            blocks = [(b, qo, qn, g)
                      for b in range(BPC) for (qo, qn) in QBL
                      for g in range(4)]
            uz_state = {}

            def s_mms(bi, i):
                b, qo, qn, g = blocks[bi]
                to = b * N
                ko, kn = KCH[i]
                h0 = 2 * g
                s_t = ps2.tile([128, 2, 512], FP, tag="s", bufs=2,
                               name=f"s_{bi}_{i}")
                for j in range(2):
                    h = h0 + j
                    hp = 32 * (h % 4)
                    hq = h // 4
                    hk = 2 + h // 4
                    nc.tensor.matmul(
                        s_t[:kn, j, :qn],
                        lhsT=qkT_sb[hp:hp + 32, hk, to + ko:to + ko + kn],
                        rhs=qkT_sb[hp:hp + 32, hq, to + qo:to + qo + qn],
                        start=True, stop=True, tile_position=(hp, 0))
                e_t = s2pool.tile([128, 2, 392], FH, tag="e",
                                  bufs=3, name=f"e_{bi}_{i}")
                nc.scalar.activation(e_t[:kn, :, :qn],
                                     s_t[:kn, :, :qn], AFT.Exp)
                return e_t

            def e2_mult(bi, i, e_t):
                b, qo, qn, g = blocks[bi]
                kn = KCH[i][1]
                # multiply by exp(bias); flat contiguous views keep the DVE
                # in 2x packed mode. High priority so the scheduler never
                # starves U/Z of e2 behind per-block finish work.
                qb4g = (qo // 392) * 4 + g
                e2_t = s2pool.tile([128, 2, 392], FH, tag="e2",
                                   bufs=ULAG + 2, name=f"e2_{bi}_{i}")
                with tc.high_priority(offset=60):
                    nc.vector.tensor_tensor(
                        e2_t[:kn].rearrange("p b q -> p (b q)"),
                        e_t[:kn].rearrange("p b q -> p (b q)"),
                        eb_sb[:kn, qb4g, i, :],
                        op=AluOpType.mult)
                return e2_t

            def z_sstep(bi, i, e2_t):
                b, qo, qn, g = blocks[bi]
                kn = KCH[i][1]
                if i == 0:
                    uz_state[bi] = {
                        "z": ps2.tile([128, 2, 512], FP, tag="z", bufs=1,
                                      name=f"z_{bi}")}
                z_ps = uz_state[bi]["z"]
                for j in range(2):
                    nc.tensor.matmul(
                        z_ps[:, j, :qn],
                        lhsT=ones_sb[:kn, :],
                        rhs=e2_t[:kn, j, :qn],
                        start=(i == 0), stop=(i == NKC - 1))
                if i == NKC - 1:
                    # reciprocal right away: releases the z banks fast
                    r_t = s2pool.tile([128, 2, 392], FP, tag="r", bufs=2,
                                      name=f"r_{bi}")
                    nc.vector.reciprocal_approx_fast(r_t[:, :, :qn],
                                                     z_ps[:, :, :qn])
                    uz_state[bi]["r"] = r_t

            def u_sstep(bi, i, e2_t):
                b, qo, qn, g = blocks[bi]
                kn = KCH[i][1]
                if i == 0:
                    uz_state[bi]["u"] = ps2.tile([128, 2, 512], FP, tag="u",
                                                 bufs=1, name=f"u_{bi}")
                u_ps = uz_state[bi]["u"]
                for j in range(2):
                    h = 2 * g + j
                    nc.tensor.matmul(
                        u_ps[:, j, :qn],
                        lhsT=v_sb[:kn, b * NKC + i, h, :],
                        rhs=e2_t[:kn, j, :qn],
                        start=(i == 0), stop=(i == NKC - 1))
                if i == NKC - 1:
                    finish_block(bi)

            def finish_block(bi):
                b, qo, qn, g = blocks[bi]
                to = b * N
                h0 = 2 * g
                st = uz_state.pop(bi)
                u_ps, r_t = st["u"], st["r"]
                d_t = s2pool.tile([128, 2, 392], FH, tag="d", bufs=1,
                                  name=f"d_{bi}")
                nc.vector.tensor_tensor(d_t[:, :, :qn], u_ps[:, :, :qn],
                                        r_t[:, :, :qn], op=AluOpType.mult)
                # hardswish: clip on DVE (4x dual-scalar), final mul on GpSimd
                t_t = s2pool.tile([128, 2, 392], FH, tag="t", bufs=1,
                                  name=f"t_{bi}")
                nc.vector.tensor_scalar(t_t[:, :, :qn], d_t[:, :, :qn],
                                        3.0, 0.0,
                                        op0=AluOpType.add, op1=AluOpType.max)
                a_t = s2pool.tile([128, 2, 392], FH, tag="a", bufs=1,
                                  name=f"a_{bi}")
                nc.vector.tensor_scalar(a_t[:, :, :qn], t_t[:, :, :qn],
                                        6.0, None, op0=AluOpType.min)
                nc.gpsimd.tensor_tensor(
                    o_sb[:, h0:h0 + 2, to + qo:to + qo + qn],
                    a_t[:, :, :qn], d_t[:, :, :qn], op=AluOpType.mult)

            # Global pipeline over per-chunk steps.
            ZLAG, ULAG = 4, 6
            seq = [(bi, i) for bi in range(len(blocks))
                   for i in range(NKC)]
            e2s = {}
            for t in range(len(seq) + ULAG):
                if t < len(seq):
                    bi, i = seq[t]
                    e_t = s_mms(bi, i)
                if t - ZLAG >= 0 and t - ZLAG < len(seq):
                    z_sstep(*seq[t - ZLAG], e2s[t - ZLAG])
                if t < len(seq):
                    e2s[t] = e2_mult(bi, i, e_t)
                if t - ULAG >= 0:
                    u_sstep(*seq[t - ULAG], e2s.pop(t - ULAG))

        # ---------- stage 3: output projection ----------
        with tc.tile_pool(name="s3", bufs=2) as s3pool, \
             tc.tile_pool(name="ps3", bufs=1, space="PSUM") as ps3:
            for (no, nn) in TB4:
                pj = [ps3.tile([128, 392], FP, tag=f"pj{c4}", bufs=1,
                               name=f"pj{c4}_{no}")
                      for c4 in range(DIMC)]
                for dhc in range(DHC):
                    for c4 in range(DIMC):
                        nc.tensor.matmul(
                            pj[c4][:, :nn],
                            lhsT=wp_sb[:, dhc, c4 * 128:(c4 + 1) * 128],
                            rhs=o_sb[:, dhc, no:no + nn],
                            start=(dhc == 0), stop=(dhc == DHC - 1))
                for c4 in range(DIMC):
                    o_st = s3pool.tile([128, 392], FH, tag="outst", bufs=4)
                    nc.scalar.activation(o_st[:, :nn], pj[c4][:, :nn],
                                         AFT.Identity,
                                         bias=bp_sb[:, c4:c4 + 1])
                    nc.sync.dma_start(out[c4 * 128:(c4 + 1) * 128, no:no + nn],
                                      o_st[:, :nn])

    # populate .instr bytes for InstISA (custom-DVE ops) — raw Bass skips this
    mybir.codegen_inst_isa_subclasses(nc)
    nsplit = _split_multi_waits(nc)
    if os.environ.get("KERNEL_DEBUG"):
        print(f"[kernel] split {nsplit} multi-wait instructions")
    return nc


def _prepare_host_inputs(x, w_qkv, qkv_g, qkv_b, qkv_m, qkv_v, ab, w_proj,
                         proj_g, proj_b, proj_m, proj_v, bias_idx):
    f32 = np.float32
    x = np.asarray(x, f32)
    w_qkv = np.asarray(w_qkv, f32)
    qkv_g = np.asarray(qkv_g, f32)
    qkv_b = np.asarray(qkv_b, f32)
    qkv_m = np.asarray(qkv_m, f32)
    qkv_v = np.asarray(qkv_v, f32)
    ab = np.asarray(ab, f32)
    w_proj = np.asarray(w_proj, f32)
    proj_g = np.asarray(proj_g, f32)
    proj_b = np.asarray(proj_b, f32)
    proj_m = np.asarray(proj_m, f32)
    proj_v = np.asarray(proj_v, f32)
    bias_idx = np.asarray(bias_idx)

    # fold qkv BN: y = (x@W^T)*s + (b - m*s)
    s = qkv_g / np.sqrt(qkv_v + EPS)
    w_f = w_qkv * s[:, None]
    b_f = qkv_b - qkv_m * s

    # channel c = h*192 + i; i<32 q (pre-scale by SCALE), <64 k, else v
    q_rows = [w_f[h * 192:h * 192 + 32] * SCALE for h in range(H)]
    k_rows = [w_f[h * 192 + 32:h * 192 + 64] for h in range(H)]
    v_rows = [w_f[h * 192 + 64:h * 192 + 192] for h in range(H)]
    q_b = [b_f[h * 192:h * 192 + 32] * SCALE for h in range(H)]
    k_b = [b_f[h * 192 + 32:h * 192 + 64] for h in range(H)]
    v_b = [b_f[h * 192 + 64:h * 192 + 192] for h in range(H)]

    w_qk = np.concatenate(q_rows + k_rows, axis=0)      # [512, 512]
    bqk = np.concatenate(q_b + k_b, axis=0)             # [512]
    w_v = np.concatenate(v_rows, axis=0)                # [1024, 512]
    bv = np.concatenate(v_b, axis=0)                    # [1024]

    wqkT = np.ascontiguousarray(w_qk.T)                 # [512 dim, 512 ch]
    wvT = np.ascontiguousarray(w_v.T)                   # [512, 1024]

    # fold proj BN + hardswish /6: P = hs6(o) @ (W*s/6)^T + (b - m*s)
    sp = proj_g / np.sqrt(proj_v + EPS)
    w_p = w_proj * sp[:, None] / 6.0
    bpv = proj_b - proj_m * sp
    wpT = np.ascontiguousarray(w_p.T)                   # [1024, 512]

    # multiplicative positional bias exp(ab[h, bias_idx]), fp16,
    # partition-contiguous: ebp[p, (qb, g, kc, j, q')]
    ebias = np.zeros((H, NKC * 128, N), np.float16)  # pad to 896 key rows
    ebias[:, :N, :] = np.exp(ab[:, bias_idx]).astype(np.float16)
    # [g, j, kc, p, qb, q'] -> [p, qb, g, kc, j, q']
    ebp = np.ascontiguousarray(
        ebias.reshape(4, 2, NKC, 128, 2, 392).transpose(3, 4, 0, 2, 1, 5)
        .reshape(128, 8 * NKC * N))

    wqkT = wqkT.astype(np.float16)
    wvT = wvT.astype(np.float16)
    wpT = wpT.astype(np.float16)

    # x transposed: [B, DIM, N] then per-core concat of its 2 batches
    xT_all = np.ascontiguousarray(x.transpose(0, 2, 1).astype(np.float16))

    in_maps = []
    for c in range(NCORES):
        xt_core = np.ascontiguousarray(
            np.concatenate([xT_all[BPC * c + b] for b in range(BPC)], axis=1))
        in_maps.append(dict(
            xT=xt_core, wqkT=wqkT, wvT=wvT, wpT=wpT,
            bqk=np.ascontiguousarray(bqk),
            bvrow=np.ascontiguousarray(
                np.broadcast_to(bv.astype(np.float16), (128, DH))),
            bp=np.ascontiguousarray(bpv), ebp=ebp,
        ))
    return in_maps


def _get_program():
    if "nc" not in _PROGRAM_CACHE:
        _PROGRAM_CACHE["nc"] = build_program()
    return _PROGRAM_CACHE["nc"]


def run(inputs: dict, trace: bool = False, trace_kwargs: dict | None = None):
    """Build+run; returns (full_output [16,784,512], BassKernelResults)."""
    nc = _get_program()
    in_maps = _prepare_host_inputs(**inputs)
    kw = {}
    if trace:
        kw = dict(trace=True, trace_cores=[0], **(trace_kwargs or {}))
    res = run_bass_kernel_spmd(nc, in_maps, core_ids=list(range(NCORES)), **kw)
    outs = []
    for c in range(NCORES):
        o = res.results[c]["out"]  # [512, 1568] fp16
        o = o.reshape(DIM, BPC, N).transpose(1, 2, 0)  # [2, 784, 512]
        outs.append(o)
    full = np.concatenate(outs, axis=0).astype(np.float32)
    return full, res


def kernel(**inputs) -> np.ndarray:
    out, _ = run(inputs, trace=False)
    return out
